# revision 6
# baseline (speedup 1.0000x reference)
"""MultiLabelSupConLoss Trainium2 kernel (8-core SPMD, Bass/Tile), v2.

Math (same as v1)
-----------------
reference computes, with l_ij = <f0_i, f0_j>/T (f0 = features[:,0,:]):
    logits_max_i = max_j over the full [2B] row of contrast similarities
    e = exp(l[:B,:B] - logits_max)
    per_row = log(sum_j e_ij) - log(sum_{j in pos(i)} e_ij)
    loss = mean over rows with >=1 positive
per_row is invariant to ANY per-row shift c_i, so we use c_i = l_ii
(the self-similarity) instead of the full-row max; this keeps exp() in
range and makes the [B:2B] half of the contrast matrix unnecessary.

The positive mask sim_ij >= 0.5 (sim = inter/(union+1e-6), integer label
counts) is equivalent to z_ij = 3*inter - rs_i - rs_j >= 1 via a single
augmented matmul over K=102 (padded to 128).

v2 engine assignment (v1 was Scalar-bound by exp + accumulator reads
+ slow heads/tails).  GPSIMD/Pool cannot touch PSUM on TRN2 and cannot
run TensorScalarPtr or free-axis reductions at all, so the elementwise
work necessarily lands on ACT + DVE only:
per (i-chunk 128 rows x 1024-column chunk) block:
    PE  : l = fTb.T @ fT -> PSUM ; z = labL.T @ labR -> PSUM (one block
          behind l, so a slow pos-stt can't stall the next l)
    ACT : e = exp(l + bias_i) -> bf16 SBUF, accum_out -> den partial
    DVE : (z >= 0.5) * e stt with accum_out -> pos partial
    POOL: input DMAs via its SWDGE ring (descgen on the idle Q7s)

DMA: one combined bf16 input tensor per core (bias fp32 bit-cast into
bf16 columns); the head block (fTb+bias) rides the scalar ring in
parallel with chunk 0 on the sync ring; everything else streams on the
sync ring in need order.  den and pos outputs ship separately.
"""

import numpy as np
import ml_dtypes

import concourse.bass as bass
import concourse.bacc as bacc
import concourse.mybir as mybir
from concourse import tile
from concourse.bass_utils import run_bass_kernel_spmd

B = 4096
D = 128
N_CORES = 8
ROWS = B // N_CORES          # 512 rows per core
ICHUNK = 128                 # rows per i-chunk (PSUM partition dim)
IC = ROWS // ICHUNK          # 4
W = 1024                     # column chunk width
NCH = B // W                 # 4 chunks
NPAIR = NCH // 2             # den reduction granularity: chunk pairs
KLAB = 128                   # 100 label dims + 2 augmentation rows + pad
TEMP = 0.07

# combined input column layout (all bf16)
C_FTB = 0                    # [0,512)    fTb block
C_BIAS = ROWS                # [512,520)  bias fp32 bitcast as 8 bf16 cols
C_LABL = ROWS + 2 * IC       # [520,1032) labL block
C_CH = 2 * ROWS + 2 * IC     # chunk pairs start
TOTC = C_CH + 2 * B          # + per chunk [fT_ch | labR_ch]

BF16 = ml_dtypes.bfloat16

_cached = None


def _chunk_cols(ch):
    """(fT columns, labR columns) of chunk ch inside the combined tensor."""
    base = C_CH + 2 * ch * W
    return slice(base, base + W), slice(base + W, base + 2 * W)


def _build_nc():
    f32 = mybir.dt.float32
    bf16 = mybir.dt.bfloat16
    nc = bacc.Bacc(
        "TRN2",
        target_bir_lowering=False,
        debug=False,
        num_devices=N_CORES,
    )

    inp_d = nc.dram_tensor("inp", [128, TOTC], bf16, kind="ExternalInput")
    den_d = nc.dram_tensor("den", [ICHUNK, IC * NCH], f32, kind="ExternalOutput")
    pos_d = nc.dram_tensor("pos", [ICHUNK, IC * NCH], f32, kind="ExternalOutput")

    act_exp = mybir.ActivationFunctionType.Exp
    alu = mybir.AluOpType

    with tile.TileContext(nc) as tc:
        with (
            tc.tile_pool(name="const", bufs=1) as cpool,
            tc.tile_pool(name="psl", bufs=2, space="PSUM") as psl,
            tc.tile_pool(name="psz", bufs=2, space="PSUM") as psz,
        ):
            inp_s = cpool.tile([128, TOTC], bf16)
            den_s = cpool.tile([ICHUNK, IC * NCH], f32)
            pos_s = cpool.tile([ICHUNK, IC * NCH], f32)
            warm = cpool.tile([ICHUNK, 512], bf16)
            tr_dve = cpool.tile([ICHUNK, W], bf16)
            # e stays fp32: the exp accumulator (den) sums the engine's
            # internal fp32 results, so the stt (pos) must read the same
            # fp32 values or den/pos disagree at bf16 rounding level.
            e_blk = [
                cpool.tile([ICHUNK, W], f32, name=f"e_blk_{i}")
                for i in range(4)
            ]

            bias_f32 = inp_s[:, C_BIAS : C_BIAS + 2 * IC].bitcast(f32)

            # --- input DMAs spread across the three DGE rings (SP/ACT
            # hardware DGE + Pool SWDGE) in need order:
            #   SP  : fT0, fT1, fT2
            #   ACT : head (fTb+bias), [table preload], labR0, labR1, labR2
            #   Pool: warm memset, labL, fT3, labR3
            f0, r0 = _chunk_cols(0)
            f1, r1 = _chunk_cols(1)
            f2, r2 = _chunk_cols(2)
            f3, r3 = _chunk_cols(3)
            scratch = cpool.tile([1, 8], f32)

            nc.gpsimd.memset(warm[:], 0.0)
            nc.vector.memset(scratch[:], 0.0)
            # preload the exp spline table immediately (the auto-inserted
            # table load otherwise waits behind the first exp's data deps)
            nc.scalar.activation(scratch[:], scratch[:], act_exp,
                                 bias=scratch[:, 0:1])
            # input loads: a single in-flight transfer moves ~55GB/s, so
            # the critical first operands are split into concurrent
            # streams across the sync + scalar + Pool-SWDGE rings.
            def _half(c):
                m = (c.start + c.stop) // 2
                return slice(c.start, m), slice(m, c.stop)
            f0a, f0b = _half(f0)
            r0a, r0b = _half(r0)
            nc.sync.dma_start(inp_s[:, f0a], inp_d[:, f0a])
            nc.scalar.dma_start(inp_s[:, 0:C_LABL], inp_d[:, 0:C_LABL])
            nc.sync.dma_start(inp_s[:, f0b], inp_d[:, f0b])
            nc.gpsimd.dma_start(inp_s[:, C_LABL:C_CH], inp_d[:, C_LABL:C_CH])
            nc.sync.dma_start(inp_s[:, r0a], inp_d[:, r0a])
            nc.scalar.dma_start(inp_s[:, r0b], inp_d[:, r0b])
            nc.sync.dma_start(inp_s[:, f1], inp_d[:, f1])
            nc.sync.dma_start(inp_s[:, r1], inp_d[:, r1])
            nc.gpsimd.dma_start(inp_s[:, f3], inp_d[:, f3])
            nc.sync.dma_start(inp_s[:, f2], inp_d[:, f2])
            nc.sync.dma_start(inp_s[:, r2], inp_d[:, r2])
            nc.gpsimd.dma_start(inp_s[:, r3], inp_d[:, r3])

            # --- PE clock warmup: sustained matmuls ramp the PE clock
            # (0.65 -> 1.2 -> 2.4 GHz after ~3us busy) while inputs stream.
            wps = psz.tile([ICHUNK, 512], f32, tag="z_ps")
            for _ in range(4):
                nc.tensor.matmul(wps[:], warm[:, :ICHUNK], warm[:])

            # --- main pipeline (ch-major blocks; z matmuls and pos-stt
            # run one block behind the l/exp chain)
            blocks = [(ch, ic) for ch in range(NCH) for ic in range(IC)]

            def _emit_l(k):
                ch, ic = blocks[k]
                fs, _ = _chunk_cols(ch)
                isl_f = slice(C_FTB + ic * ICHUNK, C_FTB + (ic + 1) * ICHUNK)
                l_ps = psl.tile([ICHUNK, W], f32)
                for h in range(W // 512):
                    nc.tensor.matmul(
                        l_ps[:, h * 512 : (h + 1) * 512],
                        inp_s[:, isl_f],
                        inp_s[:, fs.start + h * 512 : fs.start + (h + 1) * 512],
                    )
                return l_ps

            def _emit_z(k):
                ch, ic = blocks[k]
                _, rs = _chunk_cols(ch)
                isl_l = slice(C_LABL + ic * ICHUNK, C_LABL + (ic + 1) * ICHUNK)
                z_ps = psz.tile([ICHUNK, W], f32)
                for h in range(W // 512):
                    nc.tensor.matmul(
                        z_ps[:, h * 512 : (h + 1) * 512],
                        inp_s[:, isl_l],
                        inp_s[:, rs.start + h * 512 : rs.start + (h + 1) * 512],
                    )
                return z_ps

            def _e_slice(k):
                return e_blk[k % 4][:]

            def _emit_exp(k, l_ps):
                ch, ic = blocks[k]
                col = ic * NCH + ch
                nc.scalar.activation(
                    _e_slice(k),
                    l_ps[:],
                    act_exp,
                    bias=bias_f32[:, ic : ic + 1],
                    scale=1.0,
                    accum_out=den_s[:, col : col + 1],
                )

            def _emit_stt(k, z_ps):
                ch, ic = blocks[k]
                col = ic * NCH + ch
                nc.vector.scalar_tensor_tensor(
                    tr_dve[:],
                    z_ps[:],
                    0.5,
                    _e_slice(k),
                    op0=alu.is_ge,
                    op1=alu.mult,
                    accum_out=pos_s[:, col : col + 1],
                )

            # z/stt trail the l/exp chain while the labR transfers catch
            # up (skew 2), then tighten so no stt trails the last exp.
            def _skew(k):
                return 2 if k < 6 else (1 if k < 10 else 0)

            zdone = 0
            for k in range(len(blocks)):
                l_ps = _emit_l(k)
                _emit_exp(k, l_ps)
                while zdone <= k - _skew(k):
                    z_ps = _emit_z(zdone)
                    _emit_stt(zdone, z_ps)
                    zdone += 1
            while zdone < len(blocks):
                z_ps = _emit_z(zdone)
                _emit_stt(zdone, z_ps)
                zdone += 1

            # den completes with the last exp; ship it on the scalar ring
            # while the final stt drains, then pos on sync.
            nc.scalar.dma_start(den_d[:], den_s[:])
            nc.sync.dma_start(pos_d[:], pos_s[:])

    nc.compile()
    return nc, {"inp": inp_d.name, "den": den_d.name, "pos": pos_d.name}


def _get_nc():
    global _cached
    if _cached is None:
        _cached = _build_nc()
    return _cached


def _prep_inputs(features, labels):
    """Host-side shard prep: combined transposed/casted operand per core."""
    f0 = np.asarray(features)[:, 0, :].astype(np.float32)      # [B, D]
    lab = np.asarray(labels).astype(np.float32)                # [B, 100]

    s = np.float32(1.0) / np.float32(np.sqrt(np.float32(TEMP)))
    fT16 = np.ascontiguousarray((f0 * s).T).astype(BF16)       # [D, B] bf16
    # row self-similarity (= diagonal of l), from the same bf16 values
    c = (fT16.astype(np.float32) ** 2).sum(axis=0, dtype=np.float32)  # [B]

    rs = lab.sum(axis=1, dtype=np.float32)                     # [B] integers
    labT = lab.T                                               # [100, B]
    L = np.zeros((KLAB, B), dtype=np.float32)
    L[:100] = labT
    L[100] = 1.0
    L[101] = rs
    R = np.zeros((KLAB, B), dtype=np.float32)
    R[:100] = 3.0 * labT
    R[100] = -rs
    R[101] = -1.0
    L16 = L.astype(BF16)
    R16 = R.astype(BF16)

    nc, names = _get_nc()
    in_maps = []
    for core in range(N_CORES):
        blk = slice(core * ROWS, (core + 1) * ROWS)
        bias = np.ascontiguousarray(
            (-c[blk]).reshape(IC, ICHUNK).T.astype(np.float32)
        )  # [128, IC] fp32
        bias16 = bias.view(np.uint16).view(BF16)               # [128, 2*IC] raw
        inp = np.empty((128, TOTC), dtype=BF16)
        inp[:, C_FTB:C_FTB + ROWS] = fT16[:, blk]
        inp[:, C_BIAS:C_BIAS + 2 * IC] = bias16
        inp[:, C_LABL:C_LABL + ROWS] = L16[:, blk]
        for ch in range(NCH):
            fs, rs_ = _chunk_cols(ch)
            csl = slice(ch * W, (ch + 1) * W)
            inp[:, fs] = fT16[:, csl]
            inp[:, rs_] = R16[:, csl]
        in_maps.append({names["inp"]: inp})
    return nc, names, in_maps


def _fold_core(r, names):
    """Per-core den/pos row vectors from the raw output tiles."""
    dc = r[names["den"]].reshape(ICHUNK, IC, NCH).sum(axis=2, dtype=np.float32)
    pc = r[names["pos"]].reshape(ICHUNK, IC, NCH).sum(axis=2, dtype=np.float32)
    return dc.T.reshape(ROWS), pc.T.reshape(ROWS)


def _finish(results, names):
    """Host epilogue: per-row log-ratio + masked mean over 4096 rows."""
    den = np.empty(B, dtype=np.float32)
    pos = np.empty(B, dtype=np.float32)
    for core, r in enumerate(results):
        blk = slice(core * ROWS, (core + 1) * ROWS)
        den[blk], pos[blk] = _fold_core(r, names)
    has = pos > 0
    per_row = np.zeros(B, dtype=np.float32)
    per_row[has] = np.log(den[has]) - np.log(pos[has])
    count = np.float32(max(int(has.sum()), 1))
    loss = np.float32(per_row.sum(dtype=np.float32) / count)
    return np.asarray(loss, dtype=np.float32)


def kernel(features, labels):
    nc, names, in_maps = _prep_inputs(features, labels)
    res = run_bass_kernel_spmd(nc, in_maps, list(range(N_CORES)))
    return _finish(res.results, names)


def kernel_with_results(features, labels, **spmd_kwargs):
    """Like kernel() but also returns the BassKernelResults (for tracing)."""
    nc, names, in_maps = _prep_inputs(features, labels)
    res = run_bass_kernel_spmd(nc, in_maps, list(range(N_CORES)), **spmd_kwargs)
    return _finish(res.results, names), res


# revision 7
# speedup vs baseline: 1.2475x; 1.2475x over previous
"""MultiLabelSupConLoss Trainium2 kernel (8-core SPMD, Bass/Tile).

Math
----
reference computes, with l_ij = <f0_i, f0_j>/T (f0 = features[:,0,:]):
    logits_max_i = max_j over the full [2B] row of contrast similarities
    e = exp(l[:B,:B] - logits_max)
    per_row = log(sum_j e_ij) - log(sum_{j in pos(i)} e_ij)
    loss = mean over rows with >=1 positive

per_row is invariant to ANY per-row shift c_i (it cancels in the
log-difference), so instead of the full-row max we use c_i = l_ii
(the self-similarity, which dominates every row by a huge margin for
normalized-random features; using it keeps exp() in range exactly like
the reference's row max does).  This removes the need to ever compute
the second half [B:2B] of the contrast matrix: those columns only
entered through logits_max.

The positive mask sim_ij >= 0.5 with sim = inter/(union+1e-6) is
equivalent (integer label counts) to z_ij = 3*inter - rs_i - rs_j >= 1,
computed by a single augmented matmul over K=102 (padded to 128):
    lhsT rows: [labels.T ; ones ; rs ; 0...],
    rhs rows:  [3*labels.T ; -rs ; -ones ; 0...]

Sharding: data-parallel over rows; each of the 8 cores handles 512 rows
and returns per-row (den, pos) partial sums; the host does the final
log/mean (a 4096-element epilogue).

Per core device pipeline, per (i-chunk 128 rows x column chunk):
    PE : z  = labAug_blk.T @ labAug -> PSUM (bf16 in, fp32 acc)
    PE : l  = f0T_blk.T @ f0T       -> PSUM
    ACT: e  = exp(l + bias_i), accum_out -> den partial   (1 op per chunk)
    DVE: (z >= 0.5) * e,      accum_out -> pos partial    (1 fused op per chunk)
plus: exp-table preload and PE clock-warmup matmuls overlapped with the
input DMAs, column-chunked loads in need order on the fast SP DGE ring.
"""

import numpy as np
import ml_dtypes

import concourse.bass as bass
import concourse.bacc as bacc
import concourse.mybir as mybir
from concourse import tile
from concourse.bass_utils import run_bass_kernel_spmd

B = 4096
D = 128
N_CORES = 8
ROWS = B // N_CORES          # 512 rows per core
ICHUNK = 128                 # rows per i-chunk (PSUM partition dim)
IC = ROWS // ICHUNK          # 4
# column chunks: small first chunks so compute starts as soon as ~0.5MB
# of input has landed; 1024-wide steady chunks (2 PSUM banks)
CHUNKS = [512, 512, 1024, 1024, 1024]
NCH = len(CHUNKS)
CH_OFF = [sum(CHUNKS[:i]) for i in range(NCH)]
KLAB = 128                   # 100 label dims + 2 augmentation rows + pad
TEMP = 0.07

BF16 = ml_dtypes.bfloat16

_cached = None


def _build_nc():
    f32 = mybir.dt.float32
    bf16 = mybir.dt.bfloat16
    nc = bacc.Bacc(
        "TRN2",
        target_bir_lowering=False,
        debug=False,
        num_devices=N_CORES,
    )

    fT_d = nc.dram_tensor("ft_full", [D, B], bf16, kind="ExternalInput")
    fTb_d = nc.dram_tensor("ft_blk", [D, ROWS], bf16, kind="ExternalInput")
    labR_d = nc.dram_tensor("lab_full", [KLAB, B], bf16, kind="ExternalInput")
    labL_d = nc.dram_tensor("lab_blk", [KLAB, ROWS], bf16, kind="ExternalInput")
    bias_d = nc.dram_tensor("bias", [ICHUNK, IC], f32, kind="ExternalInput")
    den_d = nc.dram_tensor("den", [ICHUNK, IC * NCH], f32, kind="ExternalOutput")
    pos_d = nc.dram_tensor("pos", [ICHUNK, IC * NCH], f32, kind="ExternalOutput")

    act_exp = mybir.ActivationFunctionType.Exp

    with tile.TileContext(nc) as tc:
        with (
            tc.tile_pool(name="const", bufs=1) as cpool,
            tc.tile_pool(name="e", bufs=6) as epool,
            tc.tile_pool(name="em", bufs=4) as empool,
            tc.tile_pool(name="psl", bufs=2, space="PSUM") as psl,
            tc.tile_pool(name="psz", bufs=2, space="PSUM") as psz,
        ):
            fT_s = cpool.tile([D, B], bf16)
            fTb_s = cpool.tile([D, ROWS], bf16)
            labR_s = cpool.tile([KLAB, B], bf16)
            labL_s = cpool.tile([KLAB, ROWS], bf16)
            bias_s = cpool.tile([ICHUNK, IC], f32)
            den_s = cpool.tile([ICHUNK, IC * NCH], f32)
            pos_s = cpool.tile([ICHUNK, IC * NCH], f32)
            scratch = cpool.tile([1, 8], f32)

            # Loads in need order. The SP DGE ring (sync) delivers most
            # reliably and carries everything pipeline-critical; the ACT
            # ring only tolerates tiny transfers (bias + the lhsT block).
            # Column-chunked so compute starts after ~0.5MB, not 2MB.
            def _ch(ch):
                return slice(CH_OFF[ch], CH_OFF[ch] + CHUNKS[ch])

            nc.scalar.dma_start(bias_s[:], bias_d[:])
            nc.scalar.dma_start(fTb_s[:], fTb_d[:])
            nc.sync.dma_start(labL_s[:], labL_d[:])
            nc.sync.dma_start(labR_s[:, _ch(0)], labR_d[:, _ch(0)])
            nc.sync.dma_start(fT_s[:, _ch(0)], fT_d[:, _ch(0)])
            nc.sync.dma_start(fT_s[:, _ch(1)], fT_d[:, _ch(1)])
            nc.sync.dma_start(labR_s[:, _ch(1)], labR_d[:, _ch(1)])
            nc.sync.dma_start(fT_s[:, _ch(2)], fT_d[:, _ch(2)])
            nc.sync.dma_start(labR_s[:, _ch(2)], labR_d[:, _ch(2)])
            nc.sync.dma_start(fT_s[:, _ch(3)], fT_d[:, _ch(3)])
            nc.sync.dma_start(labR_s[:, _ch(3)], labR_d[:, _ch(3)])
            nc.sync.dma_start(fT_s[:, _ch(4)], fT_d[:, _ch(4)])
            nc.sync.dma_start(labR_s[:, _ch(4)], labR_d[:, _ch(4)])

            # pre-load the exp spline tables while input DMAs stream
            nc.vector.memset(scratch[:], 0.0)
            nc.scalar.activation(
                scratch[:], scratch[:], act_exp, bias=scratch[:, 0:1]
            )

            # warm the PE clock (1.2 -> 2.4 GHz needs ~4us of sustained
            # activity) with dummy matmuls on zeroed SBUF while inputs load
            warm = cpool.tile([ICHUNK, 512], bf16)
            nc.vector.memset(warm[:], 0.0)
            wps = psz.tile([ICHUNK, 512], f32, tag="z_ps")
            for _ in range(7):
                nc.tensor.matmul(wps[:], warm[:, :ICHUNK], warm[:])

            # column-chunk outer, row-chunk inner: only chunk 0 gates the
            # first matmul; later chunks stream in behind compute.
            for ch in range(NCH):
                w = CHUNKS[ch]
                nmm = w // 512
                for ic in range(IC):
                    isl = slice(ic * ICHUNK, (ic + 1) * ICHUNK)
                    col = ic * NCH + ch

                    l_ps = psl.tile([ICHUNK, w], f32)
                    z_ps = psz.tile([ICHUNK, w], f32)
                    for h in range(nmm):
                        jsl = slice(CH_OFF[ch] + h * 512, CH_OFF[ch] + (h + 1) * 512)
                        hsl = slice(h * 512, (h + 1) * 512)
                        nc.tensor.matmul(z_ps[:, hsl], labL_s[:, isl], labR_s[:, jsl])
                        nc.tensor.matmul(l_ps[:, hsl], fTb_s[:, isl], fT_s[:, jsl])

                    e_t = epool.tile([ICHUNK, w], f32, tag="e")
                    nc.scalar.activation(
                        e_t[:],
                        l_ps[:],
                        act_exp,
                        bias=bias_s[:, ic : ic + 1],
                        scale=1.0,
                        accum_out=den_s[:, col : col + 1],
                    )

                    em_t = empool.tile([ICHUNK, w], bf16, tag="em")
                    nc.vector.scalar_tensor_tensor(
                        em_t[:],
                        z_ps[:],
                        0.5,
                        e_t[:],
                        op0=mybir.AluOpType.is_ge,
                        op1=mybir.AluOpType.mult,
                        accum_out=pos_s[:, col : col + 1],
                    )

            # den completes with the last exp (before the last stt): ship it
            # early on the off-ring path; pos in one transfer after the last
            # accumulation (extra sync-ring issues cost more than the tail
            # overlap they buy). Host folds the NCH chunk partials per row.
            nc.scalar.dma_start(den_d[:], den_s[:])
            nc.sync.dma_start(pos_d[:], pos_s[:])

    nc.compile()
    names = {
        "fT": fT_d.name,
        "fTb": fTb_d.name,
        "labR": labR_d.name,
        "labL": labL_d.name,
        "bias": bias_d.name,
        "den": den_d.name,
        "pos": pos_d.name,
    }
    return nc, names


def _get_nc():
    global _cached
    if _cached is None:
        _cached = _build_nc()
    return _cached


def _prep_inputs(features, labels):
    """Host-side shard prep: transposed/casted operand layouts per core."""
    f0 = np.asarray(features)[:, 0, :].astype(np.float32)      # [B, D]
    lab = np.asarray(labels).astype(np.float32)                # [B, 100]

    s = np.float32(1.0) / np.float32(np.sqrt(np.float32(TEMP)))
    fT16 = np.ascontiguousarray((f0 * s).T).astype(BF16)       # [D, B] bf16
    # row self-similarity (= diagonal of l), from the same bf16 values
    c = (fT16.astype(np.float32) ** 2).sum(axis=0, dtype=np.float32)  # [B]

    rs = lab.sum(axis=1, dtype=np.float32)                     # [B] integers
    labT = lab.T                                               # [100, B]
    L = np.zeros((KLAB, B), dtype=np.float32)
    L[:100] = labT
    L[100] = 1.0
    L[101] = rs
    R = np.zeros((KLAB, B), dtype=np.float32)
    R[:100] = 3.0 * labT
    R[100] = -rs
    R[101] = -1.0
    L16 = L.astype(BF16)
    R16 = R.astype(BF16)

    nc, names = _get_nc()
    in_maps = []
    for core in range(N_CORES):
        blk = slice(core * ROWS, (core + 1) * ROWS)
        bias = np.ascontiguousarray(
            (-c[blk]).reshape(IC, ICHUNK).T.astype(np.float32)
        )
        in_maps.append(
            {
                names["fT"]: fT16,
                names["fTb"]: np.ascontiguousarray(fT16[:, blk]),
                names["labR"]: R16,
                names["labL"]: np.ascontiguousarray(L16[:, blk]),
                names["bias"]: bias,
            }
        )
    return nc, names, in_maps


def _finish(results, names):
    """Host epilogue: per-row log-ratio + masked mean over 4096 rows."""
    den = np.empty(B, dtype=np.float32)
    pos = np.empty(B, dtype=np.float32)
    for core, r in enumerate(results):
        blk = slice(core * ROWS, (core + 1) * ROWS)
        # [128, IC*NCH] chunk partials -> [128, IC] row sums -> row order
        dc = r[names["den"]].reshape(ICHUNK, IC, NCH).sum(axis=2, dtype=np.float32)
        pc = r[names["pos"]].reshape(ICHUNK, IC, NCH).sum(axis=2, dtype=np.float32)
        den[blk] = dc.T.reshape(ROWS)
        pos[blk] = pc.T.reshape(ROWS)
    has = pos > 0
    per_row = np.zeros(B, dtype=np.float32)
    per_row[has] = np.log(den[has]) - np.log(pos[has])
    count = np.float32(max(int(has.sum()), 1))
    loss = np.float32(per_row.sum(dtype=np.float32) / count)
    return np.asarray(loss, dtype=np.float32)


def kernel(features, labels):
    nc, names, in_maps = _prep_inputs(features, labels)
    res = run_bass_kernel_spmd(nc, in_maps, list(range(N_CORES)))
    return _finish(res.results, names)


def kernel_with_results(features, labels, **spmd_kwargs):
    """Like kernel() but also returns the BassKernelResults (for tracing)."""
    nc, names, in_maps = _prep_inputs(features, labels)
    res = run_bass_kernel_spmd(nc, in_maps, list(range(N_CORES)), **spmd_kwargs)
    return _finish(res.results, names), res

